# revision 15
# baseline (speedup 1.0000x reference)
"""MoE layer (top-2 of 8 experts, SwiGLU FFN) on 8 trn2 NeuronCores.

Strategy: expert parallelism. Each core owns one expert. The host computes
only the top-2 *selection* (index lists) and performs the dispatch/combine
data movement (gather tokens per expert / scatter-add partial outputs); all
floating-point math that produces output values — gate logits, top-2
softmax weights, the SwiGLU FFN — runs on device.

Device kernel (identical program on all 8 cores, per-core data):
  inputs   xt    [D, C]  gathered tokens for this expert, transposed
           gw    [D, E]  gate weights, columns rotated so own expert = col 0
           w1,w3 [D, F]  expert FFN in-projections
           w2    [F, D]  expert FFN out-projection
           valid [C]     1.0 for real tokens, 0.0 for padding
  output   yt    [D, C]  weighted expert contribution (transposed)

  per token tile (<=512 tokens):
    logitsT[8, TT] = gw.T @ xT          (PE)
    transpose to [tok, 8], top-2 softmax weight of own expert   (DVE/ACT)
    broadcast weight across partitions via DVE block-transpose + selector
    matmul                                                       (DVE/PE)
    hT[F, TT] = silu(w1.T @ xT) * (w3.T @ xT)                    (PE/ACT/DVE)
    yT[D, TT] = (w2.T)_chunks @ hT, scaled by the gate weight    (PE/DVE)
"""

import numpy as np

T, D, F, E = 8192, 1024, 4096, 8
NCORES = 8
P = 128
TOK_TILE = 512

_nc_cache: dict = {}

# "fp32r": PE multiplies in the hardware's relaxed-fp32 mode (1 cycle/row vs
# 4 for exact fp32), fp32 accumulate in PSUM. "fp32": exact but 4x slower.
MM_MODE = "fp32r"


def _build(C: int, mm_mode: str = MM_MODE):
    """Build + compile the per-core Bass program for capacity C (multiple of 128)."""
    from contextlib import ExitStack

    import concourse.tile as tile
    from concourse import bacc, mybir
    from concourse.bass import ds

    f32 = mybir.dt.float32
    dx = mybir.dt.float32r if mm_mode == "fp32r" else f32
    KD, KF = D // P, F // P
    X = mybir.AxisListType.X
    Sigmoid = mybir.ActivationFunctionType.Sigmoid
    Exp = mybir.ActivationFunctionType.Exp
    Alu = mybir.AluOpType

    nc = bacc.Bacc(
        "TRN2", target_bir_lowering=False, debug=False, num_devices=NCORES
    )
    xt = nc.dram_tensor("xt", [D, C], dx, kind="ExternalInput")
    gw = nc.dram_tensor("gw", [D, E], dx, kind="ExternalInput")
    w1 = nc.dram_tensor("w1", [D, F], dx, kind="ExternalInput")
    w3 = nc.dram_tensor("w3", [D, F], dx, kind="ExternalInput")
    w2 = nc.dram_tensor("w2", [F, D], dx, kind="ExternalInput")
    vd = nc.dram_tensor("valid", [C], f32, kind="ExternalInput")
    yt = nc.dram_tensor("yt", [D, C], f32, kind="ExternalOutput")

    with ExitStack() as ctx:
        tc = ctx.enter_context(tile.TileContext(nc))
        const = ctx.enter_context(tc.tile_pool(name="const", bufs=1))
        xp = ctx.enter_context(tc.tile_pool(name="xp", bufs=2))
        wp = ctx.enter_context(tc.tile_pool(name="wp", bufs=2))
        hp = ctx.enter_context(tc.tile_pool(name="hp", bufs=1))
        yp = ctx.enter_context(tc.tile_pool(name="yp", bufs=3))
        gp = ctx.enter_context(tc.tile_pool(name="gp", bufs=2))
        psA = ctx.enter_context(tc.tile_pool(name="psA", bufs=2, space="PSUM"))
        psG = ctx.enter_context(tc.tile_pool(name="psG", bufs=1, space="PSUM"))
        psB = ctx.enter_context(tc.tile_pool(name="psB", bufs=2, space="PSUM"))

        # constants
        gw_sb = const.tile([P, KD, E], dx)
        nc.sync.dma_start(gw_sb[:], gw[:, :].rearrange("(ko p) e -> p ko e", p=P))
        valid_sb = const.tile([P, C // P], f32)
        nc.sync.dma_start(valid_sb[:], vd[:].rearrange("(o p) -> p o", p=P))
        # selector row: picks partition 0 of the rhs in the broadcast matmul
        sel_sb = const.tile([32, P], f32)
        nc.vector.memset(sel_sb[:], 0.0)
        nc.vector.memset(sel_sb[0:1, :], 1.0)

        t0 = 0
        while t0 < C:
            TT = min(TOK_TILE, C - t0)
            S = TT // P

            x_sb = xp.tile([P, KD, TT], dx, tag="x")
            nc.sync.dma_start(
                x_sb[:], xt[:, ds(t0, TT)].rearrange("(ko p) t -> p ko t", p=P)
            )

            # ---- gating: top-2 softmax weight of own expert (col 0) ----
            lt_ps = psG.tile([E, TT], f32, tag="lt")
            for kd in range(KD):
                nc.tensor.matmul(
                    lt_ps[:],
                    gw_sb[:, kd, :],
                    x_sb[:, kd, :],
                    start=(kd == 0),
                    stop=(kd == KD - 1),
                )
            lt32 = gp.tile([32, TT], f32, tag="lt32")
            nc.vector.memset(lt32[:], 0.0)
            nc.vector.tensor_copy(lt32[0:E, :], lt_ps[:])
            # transpose to token-major: lg[tok, s, expert]
            lg = gp.tile([P, S, 32], f32, tag="lg")
            for s in range(S):
                for j in range(4):
                    nc.vector.transpose(
                        lg[ds(32 * j, 32), s], lt32[:, ds(s * P + 32 * j, 32)]
                    )
            L = lg[:, :, 0:E]
            m1 = gp.tile([P, S, 1], f32, tag="m1")
            nc.vector.reduce_max(m1[:], L, axis=X)
            d = gp.tile([P, S, E], f32, tag="d")
            nc.vector.tensor_tensor(
                d[:], L, m1[:].to_broadcast((P, S, E)), Alu.subtract
            )
            msk = gp.tile([P, S, E], f32, tag="msk")
            nc.vector.tensor_scalar(msk[:], d[:], 0.0, None, Alu.is_ge)
            nc.vector.tensor_scalar(msk[:], msk[:], -100000.0, None, Alu.mult)
            nc.vector.tensor_add(msk[:], msk[:], d[:])
            m2 = gp.tile([P, S, 1], f32, tag="m2")
            nc.vector.reduce_max(m2[:], msk[:], axis=X)
            e2 = gp.tile([P, S, 1], f32, tag="e2")
            nc.scalar.activation(e2[:], m2[:], Exp)
            den = gp.tile([P, S, 1], f32, tag="den")
            nc.vector.tensor_scalar(den[:], e2[:], 1.0, None, Alu.add)
            rec = gp.tile([P, S, 1], f32, tag="rec")
            nc.vector.reciprocal(rec[:], den[:])
            e0 = gp.tile([P, S, 1], f32, tag="e0")
            nc.scalar.activation(e0[:], d[:, :, 0:1], Exp)
            wgt = gp.tile([P, S, 1], f32, tag="wgt")
            nc.vector.tensor_mul(wgt[:], e0[:], rec[:])

            # broadcast per-token weight across partitions: wb[p, t] = w[t]
            wb_ps = psG.tile([P, TT], f32, tag="wb")
            for s in range(S):
                wcol = gp.tile([P, 32], f32, tag="wcol")
                nc.vector.memset(wcol[:, 1:32], 0.0)
                nc.vector.tensor_mul(
                    wcol[:, 0:1],
                    wgt[:, s],
                    valid_sb[:, t0 // P + s, None],
                )
                wrt = gp.tile([32, P], f32, tag="wrt")
                for j in range(4):
                    nc.vector.transpose(
                        wrt[:, ds(32 * j, 32)], wcol[ds(32 * j, 32), :]
                    )
                nc.tensor.matmul(
                    wb_ps[:, ds(s * P, P)],
                    sel_sb[:],
                    wrt[:],
                    start=True,
                    stop=True,
                )
            wb = gp.tile([P, TT], f32, tag="wb_sb")
            nc.vector.tensor_copy(wb[:], wb_ps[:])

            # ---- phase A: hT[F, TT] = silu(w1.T x) * (w3.T x) ----
            h_sb = hp.tile([P, KF, TT], dx, tag="h")
            for fp in range(KF // 2):
                w1_sb = wp.tile([P, KD, 2 * P], dx, tag="w1")
                nc.sync.dma_start(
                    w1_sb[:],
                    w1[:, ds(fp * 2 * P, 2 * P)].rearrange(
                        "(ko p) m -> p ko m", p=P
                    ),
                )
                w3_sb = wp.tile([P, KD, 2 * P], dx, tag="w3")
                nc.sync.dma_start(
                    w3_sb[:],
                    w3[:, ds(fp * 2 * P, 2 * P)].rearrange(
                        "(ko p) m -> p ko m", p=P
                    ),
                )
                for half in range(2):
                    f = fp * 2 + half
                    h1 = psA.tile([P, TT], f32, tag="h1")
                    h3 = psA.tile([P, TT], f32, tag="h3")
                    for kd in range(KD):
                        nc.tensor.matmul(
                            h1[:],
                            w1_sb[:, kd, ds(half * P, P)],
                            x_sb[:, kd, :],
                            start=(kd == 0),
                            stop=(kd == KD - 1),
                        )
                    for kd in range(KD):
                        nc.tensor.matmul(
                            h3[:],
                            w3_sb[:, kd, ds(half * P, P)],
                            x_sb[:, kd, :],
                            start=(kd == 0),
                            stop=(kd == KD - 1),
                        )
                    sg = gp.tile([P, TT], f32, tag="sg")
                    nc.scalar.activation(sg[:], h1[:], Sigmoid)
                    s1 = gp.tile([P, TT], f32, tag="s1")
                    nc.vector.tensor_mul(s1[:], sg[:], h1[:])
                    nc.vector.tensor_mul(h_sb[:, f, :], s1[:], h3[:])

            # ---- phase B: yT[D, TT] = (w2.T @ h) * wb ----
            for dm in range(KD):
                w2_sb = wp.tile([P, KF, P], dx, tag="w2")
                nc.sync.dma_start(
                    w2_sb[:],
                    w2[:, ds(dm * P, P)].rearrange("(fo p) m -> p fo m", p=P),
                )
                yps = psB.tile([P, TT], f32, tag="y")
                for fk in range(KF):
                    nc.tensor.matmul(
                        yps[:],
                        w2_sb[:, fk, :],
                        h_sb[:, fk, :],
                        start=(fk == 0),
                        stop=(fk == KF - 1),
                    )
                y_sb = yp.tile([P, TT], f32, tag="y_sb")
                nc.vector.tensor_mul(y_sb[:], yps[:], wb[:])
                nc.sync.dma_start(yt[ds(dm * P, P), ds(t0, TT)], y_sb[:])

            t0 += TT

    nc.compile()
    return nc


def _route(x: np.ndarray, gw: np.ndarray):
    """Top-2 expert selection (host; indices only — no output values)."""
    logits = x @ gw
    n = x.shape[0]
    top1 = np.argmax(logits, axis=1)
    l2 = logits.copy()
    l2[np.arange(n), top1] = -np.inf
    top2 = np.argmax(l2, axis=1)
    idx = [
        np.nonzero((top1 == e) | (top2 == e))[0].astype(np.int64)
        for e in range(gw.shape[1])
    ]
    return idx


def kernel(x, gate_w, w1, w2, w3, _trace=False, _trace_cores=None, _result_box=None):
    from concourse.bass_utils import run_bass_kernel_spmd

    x = np.ascontiguousarray(np.asarray(x, dtype=np.float32))
    gw = np.ascontiguousarray(np.asarray(gate_w, dtype=np.float32))
    w1 = np.ascontiguousarray(np.asarray(w1, dtype=np.float32))
    w2 = np.ascontiguousarray(np.asarray(w2, dtype=np.float32))
    w3 = np.ascontiguousarray(np.asarray(w3, dtype=np.float32))

    idx = _route(x, gw)
    maxn = max(len(i) for i in idx)
    C = max(P, -(-maxn // P) * P)

    key = (C, MM_MODE)
    if key not in _nc_cache:
        _nc_cache[key] = _build(C)
    nc = _nc_cache[key]

    rot = np.arange(E)
    in_maps = []
    for e in range(E):
        n = len(idx[e])
        xt = np.zeros((D, C), np.float32)
        xt[:, :n] = x[idx[e]].T
        valid = np.zeros((C,), np.float32)
        valid[:n] = 1.0
        in_maps.append(
            {
                "xt": xt,
                "gw": np.ascontiguousarray(gw[:, (rot + e) % E]),
                "w1": w1[e],
                "w3": w3[e],
                "w2": w2[e],
                "valid": valid,
            }
        )

    res = run_bass_kernel_spmd(
        nc,
        in_maps,
        core_ids=list(range(NCORES)),
        trace=_trace,
        trace_cores=_trace_cores,
    )
    if _result_box is not None:
        _result_box.append(res)

    out = np.zeros((T, D), np.float32)
    for e in range(E):
        n = len(idx[e])
        yt = np.asarray(res.results[e]["yt"])
        out[idx[e]] += yt[:, :n].T
    return out
